# revision 2
# baseline (speedup 1.0000x reference)
"""Trainium2 Bass kernel for ChunkLayer forward (padded path).

Semantics (per batch row, matching the reference's stable argsort):
  order = [positions where boundary_mask, ascending] ++ [positions where
  ~boundary_mask, ascending]
  next_hidden[j] = hidden[order[j]]           for j < K
  next_mask[j]   = j < num_boundary_tokens

Distribution: data-parallel over batch B=8 -> one row per NeuronCore,
no cross-core communication.

Per-core device algorithm:
  1. Load the row's boundary mask in a "wrapped-by-16" layout [16, L/16]
     (element e at [e%16, e//16]).
  2. Build v[e'] for e' in [0, 2L): first half  v = +(e+1) if mask else -(e+1),
     second half v = -(e+4096+1) if mask else +(e+4096+1)   (DVE).
  3. gpsimd.sparse_gather compacts the negatives out -> the first K slots hold
     (order[j]+1) or (order[j]+4096+1), encoding boundary-ness in the 4096 bit.
  4. PE-transpose the wrapped [16,128] result to natural [128,16] (PSUM),
     then DVE: idx32 = decode position, next_mask = (value <= 4096).
  5. gpsimd.indirect_dma_start gathers the K selected rows (4KB each)
     HBM->SBUF in chunks; HWDGE DMA stores them to the output, double-buffered.
"""

import functools
import numpy as np

from concourse import bass, bacc, mybir, tile
from concourse.bass_utils import run_bass_kernel_spmd

L = 4096  # sequence length per row
D = 1024  # hidden dim
B = 8  # batch == number of cores
FW = L // 16  # wrapped free width for one L-span (256)
F2 = 2 * FW  # both halves (512)
CCH = 4  # columns (=512 tokens) per payload chunk


def _build(k128: int):
    """Build + compile the per-core Bass program for K128 output rows."""
    assert k128 % 128 == 0 and 0 < k128 <= L
    nc = bacc.Bacc(None, target_bir_lowering=False, debug=False)

    hidden = nc.declare_dram_parameter("hidden", [L, D], mybir.dt.float32, isOutput=False)
    maskw = nc.declare_dram_parameter("maskw", [16, FW], mybir.dt.uint8, isOutput=False)
    out_h = nc.declare_dram_parameter("out_h", [k128, D], mybir.dt.float32, isOutput=True)
    out_m = nc.declare_dram_parameter("out_m", [k128], mybir.dt.uint8, isOutput=True)

    # Constants embedded in the NEFF.
    iota_np = (np.arange(2 * L).reshape(F2, 16).T + 1.0).astype(np.float32)
    iota_c = nc.inline_tensor(iota_np, name="iota_ep1")
    ident_c = nc.inline_tensor(np.eye(16, dtype=np.float32), name="ident16")

    Op = mybir.AluOpType

    with tile.TileContext(nc) as tc:
        with (
            tc.tile_pool(name="small", bufs=1) as sp,
            tc.tile_pool(name="psum", bufs=1, space="PSUM") as pp,
            tc.tile_pool(name="pay", bufs=3) as payp,
        ):
            mask_t = sp.tile([16, FW], mybir.dt.uint8)
            iota_t = sp.tile([16, F2], mybir.dt.float32)
            id_t = sp.tile([16, 16], mybir.dt.float32)
            nc.sync.dma_start(out=mask_t[:], in_=maskw[:])
            nc.sync.dma_start(out=iota_t[:], in_=iota_c[:])
            nc.sync.dma_start(out=id_t[:], in_=ident_c[:])

            # Sign tile: +1 where the element survives compaction, -1 where not.
            s_t = sp.tile([16, F2], mybir.dt.float32)
            nc.vector.tensor_scalar(s_t[:, 0:FW], mask_t[:], 2.0, -1.0, Op.mult, Op.add)
            nc.vector.tensor_scalar(s_t[:, FW:F2], mask_t[:], -2.0, 1.0, Op.mult, Op.add)
            v_t = sp.tile([16, F2], mybir.dt.float32)
            nc.vector.tensor_tensor(out=v_t[:], in0=s_t[:], in1=iota_t[:], op=Op.mult)

            # Stable compaction: first L outputs = order[] encoded as e+1.
            cidx_t = sp.tile([16, FW], mybir.dt.float32)
            nf_t = sp.tile([1, 1], mybir.dt.uint32)
            nc.gpsimd.sparse_gather(cidx_t[:], v_t[:], num_found=nf_t[:])

            gcols = k128 // 16  # wrapped columns holding the first k128 slots
            for g0 in range(0, gcols, 128):
                cols = min(128, gcols - g0)
                rows = cols * 16  # tokens in this group
                tok0 = g0 * 16  # first token of this group
                ps_t = pp.tile([128, 16], mybir.dt.float32)
                nc.tensor.transpose(ps_t[:cols, :], cidx_t[:, g0 : g0 + cols], id_t[:])

                # decode: value w = pos+1 (boundary) or pos+4097 (tail)
                t1_t = sp.tile([128, 16], mybir.dt.float32)
                nc.vector.tensor_scalar(
                    t1_t[:cols], ps_t[:cols], 4097.0, -4096.0, Op.is_ge, Op.mult
                )
                idx_t = sp.tile([128, 16], mybir.dt.int32)
                nc.vector.scalar_tensor_tensor(
                    idx_t[:cols], t1_t[:cols], -1.0, ps_t[:cols], Op.add, Op.add
                )
                nm_t = sp.tile([128, 16], mybir.dt.uint8)
                nc.vector.tensor_scalar(nm_t[:cols], ps_t[:cols], 4097.0, None, Op.is_lt)

                nc.sync.dma_start(
                    out=out_m[tok0 : tok0 + rows].rearrange("(p c) -> p c", c=16),
                    in_=nm_t[:cols],
                )

                outg = out_h[tok0 : tok0 + rows, :].rearrange("(p c) d -> p c d", c=16)
                for t in range(0, 16, CCH):
                    pay_t = payp.tile([128, CCH, D], mybir.dt.float32, tag="pay")
                    # The DGE path only supports one dynamic offset per
                    # partition, so issue one indirect DMA per token column.
                    for cc in range(CCH):
                        nc.gpsimd.indirect_dma_start(
                            out=pay_t[:cols, cc, :],
                            out_offset=None,
                            in_=hidden[:],
                            in_offset=bass.IndirectOffsetOnAxis(
                                ap=idx_t[:cols, t + cc : t + cc + 1], axis=0
                            ),
                        )
                    nc.sync.dma_start(out=outg[:, t : t + CCH, :], in_=pay_t[:cols])

    nc.compile()
    return nc


@functools.lru_cache(maxsize=4)
def _built(k128: int):
    return _build(k128)


def _marshal_inputs(hidden_states, boundary_mask):
    """Per-core input dicts: row b of hidden + its wrapped mask."""
    in_maps = []
    for b in range(B):
        maskw = (
            np.ascontiguousarray(
                boundary_mask[b].astype(np.uint8).reshape(FW, 16).T
            )
        )
        in_maps.append(
            {
                "hidden": np.ascontiguousarray(hidden_states[b], dtype=np.float32),
                "maskw": maskw,
            }
        )
    return in_maps


def kernel(hidden_states, boundary_mask, mask, next_max_seqlen, _trace=False):
    hidden_states = np.asarray(hidden_states)
    boundary_mask = np.asarray(boundary_mask)
    assert hidden_states.shape == (B, L, D), hidden_states.shape
    assert boundary_mask.shape == (B, L), boundary_mask.shape
    K = int(next_max_seqlen)
    assert 0 < K <= L
    k128 = ((K + 127) // 128) * 128

    nc = _built(k128)
    in_maps = _marshal_inputs(hidden_states, boundary_mask)
    res = run_bass_kernel_spmd(nc, in_maps, list(range(B)), trace=_trace)

    next_hidden = np.stack([res.results[b]["out_h"][:K] for b in range(B)])
    next_mask = np.stack([res.results[b]["out_m"][:K] for b in range(B)]).astype(bool)
    if _trace:
        return (next_hidden, next_mask), res
    return next_hidden, next_mask
